# revision 20
# baseline (speedup 1.0000x reference)
"""GQA kernel for Trainium2, sharded over 8 NeuronCores.

Sharding: data-parallel over batch (2) x tensor-parallel over kv_heads (4).
Core c = b*4 + h computes the full attention output partial
    Y_bh = softmax(causal((Q_b @ Wq_eff_h) @ (K_b @ Wk_h)^T / sqrt(dk))) @ (V_b @ Wv_h) @ Wo_h
and the host sums the 4 head partials per batch (the "all-reduce after Wo").

The GQA group-sum-before-softmax quirk folds into the weights:
    scores_h = sum_g (Q Wq_{g,h}) (K Wk_h)^T = (Q [sum_g Wq_{g,h}]) (K Wk_h)^T
so Wq_eff_h = sum_g Wq[:, (g*KV+h)*dk : ...] and each core runs standard attention.

Schedule: the kernel is HBM-wire-bound (~34MB of mandatory traffic at
358 GB/s ~ 94us vs ~82us of PE work), so everything is organized as ONE
continuous DMA stream that never stops:
    [wk][k blk0][wq][q blk0][k1][q1][k2][q2][k3][q3][wv][wo][v0][v1][v2][v3]
with y-writes overlapping the v stream from the other engine queues.
Activations are packed host-side by (seq-block, d-chunk-pair) into
contiguous 256KB pieces so each projection seq-block completes as soon as
its 2MB lands; scores(j) software-pipeline into block j+1's projection
matmuls (the always-ready proj matmuls hide the exp->rowsum dep latency);
per v-block jb: v-proj, transposes, PV(jb), normalize, Y(jb), y writes.
The causal mask is generated on device (affine_select) to save wire bytes.

Layouts (SBUF partition dim first): qT/kT/vT (dk=128, L) fp16; S^T tiles
(Lk_t=128, Lq=512) fp32 psum; row sums via ones-matmul (result replicated
across partitions == the free-dim broadcast needed to normalize O^T).
"""
import sys
sys.path.insert(0, '/opt/trn_rl_repo')
import math
import numpy as np

import concourse.bass as bass
import concourse.mybir as mybir
import concourse.tile as tile
from concourse import bacc
from concourse import bass_utils
from concourse.masks import make_identity

FP32 = mybir.dt.float32
FP16 = mybir.dt.float16

B, L, D = 2, 2048, 2048
Q_HEADS, KV_HEADS, DK, DV = 16, 4, 128, 128
GROUPS = Q_HEADS // KV_HEADS
P = 128
CH = 512                 # Lq chunk width (= seq block)
NJ = L // CH             # 4 seq blocks
NDC = D // P             # 16 contraction chunks
NPP = NDC // 4           # 4 stream pieces per block (4 chunks, 4KB lines)
SCALE = 1.0 / math.sqrt(DK)
EBIAS = -8.0 * math.log(2.0)   # exp output scaled by 2^-8; cancels in softmax
YNP = np.float16

# et_all layout: j-major flattened (j, c) score tiles
ET_OFF = {}
_off = 0
for _j in range(NJ):
    for _c in range(4 * _j + 4):
        ET_OFF[(_j, _c)] = _off
        _off += CH
ET_W = _off              # 40 * 512 fp16 = 40KB/partition


def _build():
    nc = bacc.Bacc(trn_type="TRN2")
    # activations packed host-side: rows (j, pc, p), cols (u, c)
    qt_d = nc.dram_tensor("qt", (NJ * NPP * P, 4 * CH), FP16, kind="ExternalInput")
    kt_d = nc.dram_tensor("kt", (NJ * NPP * P, 4 * CH), FP16, kind="ExternalInput")
    vt_d = nc.dram_tensor("vt", (NJ * NPP * P, 4 * CH), FP16, kind="ExternalInput")
    # weights pre-packed on host to the SBUF image: (128, NDC*dk)
    wq_d = nc.dram_tensor("wq", (P, NDC * DK), FP16, kind="ExternalInput")
    wk_d = nc.dram_tensor("wk", (P, NDC * DK), FP16, kind="ExternalInput")
    wv_d = nc.dram_tensor("wv", (P, NDC * DV), FP16, kind="ExternalInput")
    wo_d = nc.dram_tensor("wo", (DV, D), FP16, kind="ExternalInput")
    y_d = nc.dram_tensor("y", (L, D), FP16, kind="ExternalOutput")

    with tile.TileContext(nc) as tc:
        with (
            tc.tile_pool(name="const", bufs=1) as const,
            tc.tile_pool(name="wpool", bufs=1) as wpool,
            tc.tile_pool(name="xs", bufs=16) as xs,
            tc.tile_pool(name="proj", bufs=1) as proj,
            tc.tile_pool(name="yevp", bufs=4) as yevp,
            tc.tile_pool(name="ps", bufs=1, space="PSUM") as ps,
        ):
            # first touch of the wire: wk, so the k stream can start computing
            wk_sb = wpool.tile([P, NDC * DK], FP16, tag="wk", name="wk_sb")
            nc.scalar.dma_start(wk_sb[:], wk_d[:])

            # consts on gpsimd (off the critical engines)
            ident = const.tile([P, P], FP16)
            make_identity(nc, ident[:])
            ones = const.tile([P, P], FP16)
            nc.gpsimd.memset(ones[:], 1.0)
            ebias = const.tile([P, 1], FP32)
            nc.gpsimd.memset(ebias[:], EBIAS)
            maskt = const.tile([P, NJ * CH], FP16)
            for d in range(4):
                blk = maskt[:, d * CH:(d + 1) * CH]
                nc.gpsimd.memset(blk, 1.0)
                # keep 1.0 where x - p - 128*d >= 0 (k <= q), else 0
                nc.gpsimd.affine_select(
                    out=blk, in_=blk, pattern=[[1, CH]],
                    compare_op=mybir.AluOpType.is_ge, fill=0.0,
                    base=-P * d, channel_multiplier=-1)

            kT = proj.tile([P, L], FP16, tag="kT")
            qT = proj.tile([P, L], FP16, tag="qT")
            vT = proj.tile([P, L], FP16, tag="vT")
            v_nat = proj.tile([P, L], FP16, tag="v_nat")
            oT = proj.tile([P, L], FP16, tag="oT")
            et_all = proj.tile([P, ET_W], FP16, tag="et_all")
            rinv_all = proj.tile([P, NJ * CH], FP32, tag="rinv_all")

            def stream_block(xd, w_sb, dst, j, steps=None, quota=0,
                             dma_eng=None):
                """Stream one 2MB seq-block and contract it with w_sb;
                advance `quota` score-steps spread across the pieces."""
                if dma_eng is None:
                    dma_eng = nc.sync
                acc = ps.tile([P, CH], FP32, tag="acc", name="acc", bufs=2)
                for pc in range(NPP):
                    xt = xs.tile([P, 4 * CH], FP16, tag="xt", name="xt", bufs=12)
                    r0 = (j * NPP + pc) * P
                    dma_eng.dma_start(xt[:], xd[r0:r0 + P, :])
                    for u in range(4):
                        dc = 4 * pc + u
                        nc.tensor.matmul(
                            acc[:], w_sb[:, dc * P:(dc + 1) * P],
                            xt[:, u * CH:(u + 1) * CH],
                            start=(dc == 0), stop=(dc == NDC - 1))
                        if steps is not None and u % 2 == 1:
                            take = ((pc * 2 + (u // 2) + 1) * quota
                                    ) // (2 * NPP) - (
                                    (pc * 2 + (u // 2)) * quota) // (2 * NPP)
                            for _ in range(take):
                                next(steps, None)
                nc.vector.tensor_copy(dst[:, j * CH:(j + 1) * CH], acc[:])

            rreps = {}

            def score_gen(items):
                """One PE step per yield over the global score item list, with
                a 2-deep S^T pipeline so interleaved proj matmuls hide the
                exp dependency latency."""
                n = len(items)
                sts = {}

                def emit_st(i):
                    j, c = items[i]
                    st = ps.tile([P, CH], FP32, tag="mm", name="st", bufs=3)
                    nc.tensor.matmul(st[:], kT[:, c * P:(c + 1) * P],
                                     qT[:, j * CH:(j + 1) * CH],
                                     start=True, stop=True)
                    sts[i] = st

                emit_st(0)
                yield
                if n > 1:
                    emit_st(1)
                yield
                for i in range(n):
                    j, c = items[i]
                    st = sts.pop(i)
                    et = et_all[:, ET_OFF[(j, c)]:ET_OFF[(j, c)] + CH]
                    nc.scalar.activation(et, st[:],
                                         mybir.ActivationFunctionType.Exp,
                                         bias=ebias[:], scale=SCALE)
                    d = c - 4 * j
                    if d >= 0:   # diagonal tile: zero out k > q
                        nc.vector.tensor_mul(et, et, maskt[:, d * CH:(d + 1) * CH])
                    if i + 2 < n:
                        emit_st(i + 2)
                    if c == 0:
                        rreps[j] = ps.tile([P, CH], FP32, tag="accum",
                                           name=f"rrep{j}", bufs=2)
                    nc.tensor.matmul(rreps[j][:], ones[:], et,
                                     start=(c == 0), stop=(c == 4 * j + 3))
                    if c == 4 * j + 3:
                        nc.vector.reciprocal_approx_fast(
                            rinv_all[:, j * CH:(j + 1) * CH], rreps[j][:])
                    yield

            def drain(steps):
                for _ in steps:
                    pass

            wq_sb = wpool.tile([P, NDC * DK], FP16, tag="wq", name="wq_sb")
            wv_sb = wpool.tile([P, NDC * DV], FP16, tag="wv", name="wv_sb")
            wo_sb = wpool.tile([DV, D], FP16, tag="wo", name="wo_sb")

            def park(j):
                """DMA a q seq-block into SBUF without projecting it yet."""
                tiles = []
                for pc in range(NPP):
                    qp = xs.tile([P, 4 * CH], FP16, tag="qp", name="qp", bufs=8)
                    r0 = (j * NPP + pc) * P
                    nc.sync.dma_start(qp[:], qt_d[r0:r0 + P, :])
                    tiles.append(qp)
                return tiles

            def proj_parked(tiles, j):
                acc = ps.tile([P, CH], FP32, tag="acc", name="acc", bufs=2)
                for pc in range(NPP):
                    for u in range(4):
                        dc = 4 * pc + u
                        nc.tensor.matmul(
                            acc[:], wq_sb[:, dc * P:(dc + 1) * P],
                            tiles[pc][:, u * CH:(u + 1) * CH],
                            start=(dc == 0), stop=(dc == NDC - 1))
                nc.vector.tensor_copy(qT[:, j * CH:(j + 1) * CH], acc[:])

            def transp_block(b):
                tp = ps.tile([P, CH], FP16, tag="tp", name="tp", bufs=1)
                for i in range(4):
                    c = 4 * b + i
                    nc.tensor.transpose(tp[:, i * P:(i + 1) * P],
                                        vT[:, c * P:(c + 1) * P], ident[:])
                nc.vector.tensor_copy(v_nat[:, 4 * b * P:(4 * b + 4) * P], tp[:])

            def post_gen(b):
                """Post-stream work for v-block b (transpose, PV, normalize,
                Y, y write) as PE quanta, driveable from the next v-stream."""
                transp_block(b)
                yield
                nn = 4 * b + 4
                pv = ps.tile([P, CH], FP32, tag="accum", name=f"pv{b}", bufs=2)
                # last block runs undriven: column-split the PV accumulation so
                # each Y row-tile starts as soon as its 128 columns stop,
                # pipelining the tail instead of serializing pv -> norm -> Y
                groups = ([(t * P, P) for t in range(4)] if b == NJ - 1
                          else [(0, CH)])
                for g0, gw in groups:
                    for c in range(nn):
                        nc.tensor.matmul(
                            pv[:, g0:g0 + gw], v_nat[:, c * P:(c + 1) * P],
                            et_all[:, ET_OFF[(b, c)] + g0:
                                   ET_OFF[(b, c)] + g0 + gw],
                            start=(c == 0), stop=(c == nn - 1))
                        if c % 2 == 1:
                            yield
                    nc.vector.tensor_mul(
                        oT[:, b * CH + g0:b * CH + g0 + gw], pv[:, g0:g0 + gw],
                        rinv_all[:, b * CH + g0:b * CH + g0 + gw])
                    yield
                    for t in range(g0 // P, (g0 + gw) // P):
                        lq0 = b * CH + t * P
                        yv = yevp.tile([P, D], FP16, tag="yev", name="yev",
                                       bufs=3)
                        for dch in range(4):
                            yps = ps.tile([P, CH], FP32, tag="mm", name="yps",
                                          bufs=3)
                            nc.tensor.matmul(yps[:], oT[:, lq0:lq0 + P],
                                             wo_sb[:, dch * CH:(dch + 1) * CH],
                                             start=True, stop=True)
                            dst = yv[:, dch * CH:(dch + 1) * CH]
                            if dch % 2 == 0:
                                nc.vector.tensor_copy(dst, yps[:])
                            else:
                                nc.scalar.copy(dst, yps[:])
                            yield
                        nc.scalar.dma_start(y_d[lq0:lq0 + P, :], yv[:])

            # --- schedule: k/q blocks interleaved with scores(j) driven in
            #     block j+1's streams; then per v-block: proj, transpose,
            #     PV(b), normalize, Y(b), y write (v last = shortest tail) ---
            stream_block(kt_d, wk_sb, kT, 0)
            nc.sync.dma_start(wq_sb[:], wq_d[:])
            stream_block(qt_d, wq_sb, qT, 0)
            for j in range(1, NJ):
                gen = score_gen([(j - 1, c) for c in range(4 * j)])
                stream_block(kt_d, wk_sb, kT, j, gen, quota=(4 * j + 2 + 1) // 2)
                stream_block(qt_d, wq_sb, qT, j, gen, quota=(4 * j + 2) // 2)
                drain(gen)
            nc.sync.dma_start(wv_sb[:], wv_d[:])
            nc.sync.dma_start(wo_sb[:], wo_d[:])

            import itertools
            gen3 = score_gen([(3, c) for c in range(16)])
            stream_block(vt_d, wv_sb, vT, 0, gen3, quota=10)
            g1 = itertools.chain(gen3, post_gen(0))
            stream_block(vt_d, wv_sb, vT, 1, g1, quota=16)
            drain(g1)
            g2 = post_gen(1)
            stream_block(vt_d, wv_sb, vT, 2, g2, quota=18)
            drain(g2)
            g3 = post_gen(2)
            stream_block(vt_d, wv_sb, vT, 3, g3, quota=20)
            drain(g3)
            drain(post_gen(3))
    nc.compile()
    return nc


_NC = None


def _get_nc():
    global _NC
    if _NC is None:
        _NC = _build()
    return _NC


def _pack_w(w):
    """(D, dk) fp32 -> SBUF image (128, NDC*dk): out[p, dc*dk+m] = w[dc*128+p, m]"""
    return np.ascontiguousarray(
        w.reshape(NDC, P, -1).transpose(1, 0, 2).reshape(P, -1)).astype(np.float16)


def _pack_act(Xb):
    """(L, D) fp32 -> (NJ*NPP*P, 4*CH) f16, rows (j, pc, p), cols (u, c)."""
    Xt = np.asarray(Xb, np.float32).T                # (D, L)
    t = Xt.reshape(NPP, 4, P, NJ, CH)                # [pc, u, p, j, c]
    t = t.transpose(3, 0, 2, 1, 4)                   # [j, pc, p, u, c]
    return np.ascontiguousarray(t).reshape(NJ * NPP * P, 4 * CH).astype(np.float16)


def _make_in_maps(Q, K, V, Wq, Wk, Wv, Wo):
    f16 = np.float16
    # fold GQA group sum into Wq: head = g*KV_HEADS + h
    Wq_eff = np.asarray(Wq, np.float32).reshape(D, GROUPS, KV_HEADS, DK).sum(axis=1)
    acts = {}
    for b in range(B):
        acts[b] = {
            "qt": _pack_act(Q[b]),
            "kt": _pack_act(K[b]),
            "vt": _pack_act(V[b]),
        }
    Wk32, Wv32 = np.asarray(Wk, np.float32), np.asarray(Wv, np.float32)
    Wo32 = np.asarray(Wo, np.float32)
    in_maps = []
    for c in range(8):
        b, h = divmod(c, KV_HEADS)
        in_maps.append({
            **acts[b],
            "wq": _pack_w(Wq_eff[:, h, :]),
            "wk": _pack_w(Wk32[:, h * DK:(h + 1) * DK]),
            "wv": _pack_w(Wv32[:, h * DV:(h + 1) * DV]),
            "wo": Wo32[h * DV:(h + 1) * DV, :].astype(f16),
        })
    return in_maps


def _gather(results):
    Y = np.zeros((B, L, D), np.float32)
    for c in range(8):
        Y[c // KV_HEADS] += results[c]["y"].astype(np.float32)
    return Y


def kernel(Q, K, V, Wq, Wk, Wv, Wo):
    nc = _get_nc()
    in_maps = _make_in_maps(Q, K, V, Wq, Wk, Wv, Wo)
    res = bass_utils.run_bass_kernel_spmd(nc, in_maps, core_ids=list(range(8)))
    return _gather(res.results)


def _install_ntff_hook():
    """The agent image's antenv lacks axon_hooks; synthesize it so
    trace=True can reach the NTFF profiler in libaxon_pjrt.so."""
    import types
    import antenv
    if hasattr(antenv, "axon_hooks"):
        return
    mod = types.ModuleType("antenv.axon_hooks")
    _h = [None]
    mod.set_axon_ntff_profile_hook = lambda h: _h.__setitem__(0, h)
    mod.get_axon_ntff_profile_hook = lambda: _h[0]
    sys.modules["antenv.axon_hooks"] = mod
    antenv.axon_hooks = mod
    from trn_agent_boot.trn_boot import _ntff_profile_via_ctypes
    mod.set_axon_ntff_profile_hook(_ntff_profile_via_ctypes("/opt/axon/libaxon_pjrt.so"))


def kernel_traced(Q, K, V, Wq, Wk, Wv, Wo):
    """Like kernel() but profiles; returns (output, BassKernelResults)."""
    _install_ntff_hook()
    nc = _get_nc()
    in_maps = _make_in_maps(Q, K, V, Wq, Wk, Wv, Wo)
    res = bass_utils.run_bass_kernel_spmd(nc, in_maps, core_ids=list(range(8)),
                                          trace=True)
    return _gather(res.results), res
